# revision 13
# baseline (speedup 1.0000x reference)
"""Trainium2 Bass kernel for a 2-layer GCN classifier (nn_GCNClassifier).

Reference computation (all f32):
    h1 = relu(adj1 @ x @ W1 + b1) + relu(adj2 @ x @ W1 + b1)   # [8192, 64]
    h2 = relu(adj1 @ h1 @ W2 + b2) + relu(adj2 @ h1 @ W2 + b2) # [8192, 16]

Sharding: 1D row partition of adj1/adj2 across 8 cores (1024 output rows per
core). Each core receives its adjacency row-shard PRE-TRANSPOSED on the host
(adj[rows, :].T, shape [8192, 1024], contiguous) so the contraction index
lands on the SBUF partition dim and every DMA line is 4KB contiguous.

On-chip layout is feature-major ("transposed space"): aggregates are computed
as aggT[f, m] = sum_k x[k, f] * adjT[k, m] with the tiny feature block as the
stationary matmul operand and the streaming adjacency as the moving operand.
Layer 2 uses associativity: adj @ (h1 @ W2), so only [8192, 16] crosses cores
via AllGather. Traffic per core = 2 layers x 2 adjacencies x 32MB = 128MB,
which sets the memory roofline (~355-420us at the achievable DMA rate).

Engine split: the sync engine issues ONLY the streaming adjacency loads (so
its queue never head-of-line blocks on the inter-layer AllGather); gpsimd
issues all small DMAs (constants, g bounce, output stores).
"""

import numpy as np

import concourse.bacc as bacc
import concourse.mybir as mybir
import concourse.tile as tile
from concourse.bass_utils import run_bass_kernel_spmd
from concourse.masks import make_identity

N = 8192
IN_DIM, HID_DIM, OUT_DIM = 32, 64, 16
N_CORES = 8
ROWS = N // N_CORES          # 1024 output rows per core
KBLK = 128                   # contraction block (SBUF partition dim)
KMERGE = 4                   # k-blocks fetched per DMA (2MB, 16KB lines)
NKB = N // KBLK              # 64 contraction blocks
NKG = NKB // KMERGE          # 32 merged DMA groups
MCHUNK = 512                 # moving free-dim per matmul (fp32 max)
NMC = ROWS // MCHUNK         # 2 m-chunks per core
F32 = mybir.dt.float32
F32R = mybir.dt.float32r     # single-pass PE fp32 (fast at N>=256)
BF16 = mybir.dt.bfloat16
RELU = mybir.ActivationFunctionType.Relu

import os as _os
FP16 = mybir.dt.float16
WIRE = _os.environ.get("GCN_WIRE", "fp16")   # f32r | fp16 | bf16
USE_BF16 = WIRE in ("bf16", "fp16")          # half-width wire formats
ADJ_DT = {"f32r": F32R, "fp16": FP16, "bf16": BF16}[WIRE]
ADJ_BUFS = 18 if USE_BF16 else 8   # prefetch depth (slots are 1MB at half-width wire)


def _build_program():
    nc = bacc.Bacc(
        "TRN2", target_bir_lowering=False, debug=False, num_devices=N_CORES
    )
    a1t = nc.dram_tensor("a1t", [NKG, KBLK, KMERGE, ROWS], ADJ_DT, kind="ExternalInput")
    a2t = nc.dram_tensor("a2t", [NKG, KBLK, KMERGE, ROWS], ADJ_DT, kind="ExternalInput")
    featb = nc.dram_tensor("featb", [KBLK, NKB, IN_DIM], ADJ_DT, kind="ExternalInput")
    w1 = nc.dram_tensor("w1", [IN_DIM, HID_DIM], F32, kind="ExternalInput")
    b1 = nc.dram_tensor("b1", [HID_DIM, 1], F32, kind="ExternalInput")
    w2 = nc.dram_tensor("w2", [HID_DIM, OUT_DIM], F32, kind="ExternalInput")
    b2 = nc.dram_tensor("b2", [OUT_DIM, 1], F32, kind="ExternalInput")
    out = nc.dram_tensor("out", [ROWS, OUT_DIM], F32, kind="ExternalOutput")

    with tile.TileContext(nc) as tc:
        _kernel_body(nc, tc, a1t, a2t, featb, w1, b1, w2, b2, out)
    nc.compile()
    return nc


def _aggregate(nc, adjp, psp, adj_drams, lhs_blocks, fdim, tag):
    """aggT[a][f, m] = sum_k lhs[k, f] * adjT[a][k, m] for both adjacencies.

    lhs_blocks(kb) -> stationary [128, fdim] SBUF AP for contraction block kb.
    Returns 2x NMC psum tiles [fdim, MCHUNK] (accumulated over all 64 blocks).
    """
    agg_ps = [
        [psp.tile([fdim, MCHUNK], F32, tag=f"acc{ai}{mc}", name=f"{tag}{ai}{mc}")
         for mc in range(NMC)]
        for ai in range(2)
    ]
    dma_engines = (nc.sync, nc.scalar)   # one HWDGE generator per adjacency
    for kg in range(NKG):
        for ai, adj in enumerate(adj_drams):
            at = adjp.tile(
                [KBLK, KMERGE, ROWS], ADJ_DT, tag="adj",
                name=f"{tag}_adj{ai}_{kg}",
            )
            dma_engines[ai].dma_start(at[:], adj[kg])
            for t in range(KMERGE):
                kb = kg * KMERGE + t
                lhs = lhs_blocks(kb)
                for mc in range(NMC):
                    nc.tensor.matmul(
                        agg_ps[ai][mc][:],
                        lhs,
                        at[:, t, mc * MCHUNK:(mc + 1) * MCHUNK],
                        start=(kb == 0),
                        stop=(kb == NKB - 1),
                    )
    return agg_ps


def _kernel_body(nc, tc, a1t, a2t, featb, w1, b1, w2, b2, out):
    with (
        tc.tile_pool(name="const", bufs=1) as constp,
        tc.tile_pool(name="adj", bufs=ADJ_BUFS) as adjp,
        tc.tile_pool(name="work", bufs=1) as workp,
        tc.tile_pool(name="psum", bufs=1, space="PSUM") as psp,
        tc.tile_pool(name="dram", bufs=1, space="DRAM") as dramp,
    ):
        # --- constants; features are pre-blocked on the host so this is
        # one dense 1MB DMA (it leads the sync queue ahead of the adj stream)
        xb = constp.tile([KBLK, NKB, IN_DIM], ADJ_DT)   # features, k-blocked
        nc.sync.dma_start(xb[:], featb[:])
        w1_sb = constp.tile([IN_DIM, HID_DIM], F32)
        nc.gpsimd.dma_start(w1_sb[:], w1[:])
        b1_sb = constp.tile([HID_DIM, 1], F32)
        nc.gpsimd.dma_start(b1_sb[:], b1[:])
        w2_sb = constp.tile([HID_DIM, OUT_DIM], F32)
        nc.gpsimd.dma_start(w2_sb[:], w2[:])
        b2_sb = constp.tile([OUT_DIM, 1], F32)
        nc.gpsimd.dma_start(b2_sb[:], b2[:])
        ident = constp.tile([OUT_DIM, OUT_DIM], F32)
        make_identity(nc, ident[:])

        # --- layer 1: aggT = (adjT)^T-contract with x blocks ---
        agg_ps = _aggregate(
            nc, adjp, psp, (a1t, a2t), lambda kb: xb[:, kb, :], IN_DIM, "l1"
        )
        agg_sb = [
            workp.tile([IN_DIM, ROWS], F32, name=f"aggsb{ai}") for ai in range(2)
        ]
        for ai in range(2):
            for mc in range(NMC):
                nc.vector.tensor_copy(
                    agg_sb[ai][:, mc * MCHUNK:(mc + 1) * MCHUNK],
                    agg_ps[ai][mc][:],
                )

        # z1T = W1^T @ aggT ; h1T = relu(z1T + b1) summed over branches
        h1_parts = [
            workp.tile([HID_DIM, ROWS], F32, name=f"h1p{ai}") for ai in range(2)
        ]
        for ai in range(2):
            for mc in range(NMC):
                z_ps = psp.tile([HID_DIM, MCHUNK], F32, tag="zz", bufs=2)
                nc.tensor.matmul(
                    z_ps[:],
                    w1_sb[:],
                    agg_sb[ai][:, mc * MCHUNK:(mc + 1) * MCHUNK],
                    start=True,
                    stop=True,
                )
                nc.scalar.activation(
                    h1_parts[ai][:, mc * MCHUNK:(mc + 1) * MCHUNK],
                    z_ps[:],
                    RELU,
                    bias=b1_sb[:],
                )
        h1T = workp.tile([HID_DIM, ROWS], F32)
        nc.vector.tensor_add(h1T[:], h1_parts[0][:], h1_parts[1][:])

        # g = h1 @ W2, accumulated into a k-blocked [128, 8, 16] tile so the
        # inter-core exchange uses dense 512B-per-partition transfers only
        nloc = ROWS // KBLK                              # 8 local k-blocks
        g_sb = workp.tile([KBLK, nloc, OUT_DIM], ADJ_DT)
        for i in range(nloc):
            g_ps = psp.tile([KBLK, OUT_DIM], F32, tag="gg", bufs=2)
            nc.tensor.matmul(
                g_ps[:],
                h1T[:, i * KBLK:(i + 1) * KBLK],
                w2_sb[:],
                start=True,
                stop=True,
            )
            nc.vector.tensor_copy(g_sb[:, i, :], g_ps[:])
        g_loc = dramp.tile([KBLK, nloc * OUT_DIM], ADJ_DT)
        gflat = g_sb[:].rearrange("p j o -> p (j o)")
        for q in range(4):
            w = nloc * OUT_DIM // 4
            nc.gpsimd.dma_start(
                g_loc[:, q * w:(q + 1) * w], gflat[:, q * w:(q + 1) * w]
            )

        # AllGather blocked g: [128, 128] per core -> [1024, 128]
        g_cat = dramp.tile([N_CORES * KBLK, nloc * OUT_DIM], ADJ_DT,
                           addr_space="Shared")
        nc.gpsimd.collective_compute(
            "AllGather",
            mybir.AluOpType.bypass,
            replica_groups=[list(range(N_CORES))],
            ins=[g_loc.opt()],
            outs=[g_cat.opt()],
        )
        gb = constp.tile([KBLK, NKB, OUT_DIM], ADJ_DT)  # g, k-blocked
        for c in range(N_CORES):
            nc.gpsimd.dma_start(
                gb[:, c * nloc:(c + 1) * nloc, :],
                g_cat[c * KBLK:(c + 1) * KBLK, :]
                .rearrange("p (j o) -> p j o", j=nloc),
            )

        # --- layer 2: agg2T = contract adjT with g blocks (reuses acc tags) ---
        agg2_ps = _aggregate(
            nc, adjp, psp, (a1t, a2t), lambda kb: gb[:, kb, :], OUT_DIM, "l2"
        )

        # h2T = relu(agg2T + b2) summed over branches; pipelined per m-chunk
        # so transposes/stores start as soon as each chunk's add lands
        h2_parts = [
            workp.tile([OUT_DIM, ROWS], F32, name=f"h2p{ai}") for ai in range(2)
        ]
        h2T = workp.tile([OUT_DIM, ROWS], F32)
        nblk = MCHUNK // KBLK
        for mc in range(NMC):
            sl = slice(mc * MCHUNK, (mc + 1) * MCHUNK)
            for ai in range(2):
                nc.scalar.activation(
                    h2_parts[ai][:, sl], agg2_ps[ai][mc][:], RELU, bias=b2_sb[:]
                )
            nc.vector.tensor_add(
                h2T[:, sl], h2_parts[0][:, sl], h2_parts[1][:, sl]
            )
            for j in range(nblk):
                i = mc * nblk + j
                t_ps = psp.tile([KBLK, OUT_DIM], F32, tag="gg", bufs=2)
                nc.tensor.transpose(
                    t_ps[:], h2T[:, i * KBLK:(i + 1) * KBLK], ident[:]
                )
                o_sb = workp.tile([KBLK, OUT_DIM], F32, tag="osb", bufs=2)
                nc.vector.tensor_copy(o_sb[:], t_ps[:])
                nc.gpsimd.dma_start(out[i * KBLK:(i + 1) * KBLK, :], o_sb[:])


_NC_CACHE = None


def _get_nc():
    global _NC_CACHE
    if _NC_CACHE is None:
        _NC_CACHE = _build_program()
    return _NC_CACHE


def _shard_inputs(inputs):
    if WIRE == "bf16":
        import ml_dtypes
        wire_np = ml_dtypes.bfloat16
    elif WIRE == "fp16":
        wire_np = np.float16
    else:
        wire_np = np.float32
    adj1 = np.asarray(inputs["adj1"], dtype=np.float32)
    adj2 = np.asarray(inputs["adj2"], dtype=np.float32)
    feat = np.asarray(inputs["features"], dtype=np.float32)
    featb = np.ascontiguousarray(
        feat.reshape(NKB, KBLK, IN_DIM).swapaxes(0, 1)
    ).astype(wire_np)
    w1 = np.ascontiguousarray(inputs["W1"], dtype=np.float32)
    b1 = np.ascontiguousarray(inputs["b1"], dtype=np.float32).reshape(HID_DIM, 1)
    w2 = np.ascontiguousarray(inputs["W2"], dtype=np.float32)
    b2 = np.ascontiguousarray(inputs["b2"], dtype=np.float32).reshape(OUT_DIM, 1)
    in_maps = []
    for c in range(N_CORES):
        rows = slice(c * ROWS, (c + 1) * ROWS)
        # blocked-transposed: [kg, p, t, m] = adj[c*ROWS + m, kg*KM*128 + t*128 + p]
        def blockT(a):
            return np.ascontiguousarray(
                a[rows, :]
                .reshape(ROWS, NKG, KMERGE, KBLK)
                .transpose(1, 3, 2, 0)
            ).astype(wire_np, copy=False) if not USE_BF16 else (
                a[rows, :]
                .reshape(ROWS, NKG, KMERGE, KBLK)
                .transpose(1, 3, 2, 0)
                .astype(wire_np)
            )
        in_maps.append({
            "a1t": blockT(adj1),
            "a2t": blockT(adj2),
            "featb": featb,
            "w1": w1,
            "b1": b1,
            "w2": w2,
            "b2": b2,
        })
    return in_maps


def _run(inputs, trace=False, trace_cores=None, stitch_traces=False):
    nc = _get_nc()
    in_maps = _shard_inputs(inputs)
    res = run_bass_kernel_spmd(
        nc,
        in_maps,
        core_ids=list(range(N_CORES)),
        trace=trace,
        trace_cores=trace_cores,
        stitch_traces=stitch_traces,
    )
    full = np.concatenate(
        [res.results[c]["out"] for c in range(N_CORES)], axis=0
    ).astype(np.float32)
    return full, res


def kernel(**inputs):
    full, _ = _run(inputs, trace=False)
    return full
